# revision 25
# baseline (speedup 1.0000x reference)
"""Batchelor GPU-NUFFT forward operator on 8 Trainium2 NeuronCores.

Math (per timepoint t):
    warped  = bilinear_warp(image, flow[..., t])
    coil    = csm * warped                                  [Nc,Nx,Ny]
    out_t[c,s] = sum_{x,y} coil[c,x,y] exp(-2pi i (kx_s (x-64) + ky_s (y-64)))
    out     = sum_t out_t                                   [Nc,NS] complex64

Sharding: 8 cores = 4 timepoints x 2 sample-halves (4096 samples each).
Host unshard: sum the 4 timepoint partials per half, concat halves.

Device pipeline (per core):
  * warp: host provides the bf16 corner table (DRAM, row (y0*128+x0) holds the
    4 bilinear corners of real+imag), int16 gather indices in the SWDGE
    wrapped layout, and the 4 bilinear weight planes. 16 dma_gather ops land
    the corners directly in [x, y] layout (slot i = y*128 + x); the combine
    and the coil pack run per 4-gather quarter in the gather shadow.
  * NUFFT: Khatri-Rao split y = yo*8 + yi. Per 512-sample chunk, 32
    accumulating bf16 matmuls build PSUM partials Pr = Re(sum coil e^{-iA}),
    Pi = Im(...) directly (stationary blocks Cr | Ci | -Ci make the +- signs
    accumulate in PSUM). The outer phase e^{-iB} is 4 elementwise products,
    folded to 8 coils by +-selector matmuls.
  * trig: phases are range-reduced with custom DVE ops (PHASE_WRAP fuses the
    a0 = kx*(x-64) - 64*ky wrap to one op; ADD_TT_WRAP fuses each chain step
    m2_yi = wrap(m2_parent + ky2^j) with a log-depth parent tree; ABS_SUB
    preps cos args as |m|-1/4 since the ACT Sin spline is only valid on
    [-pi, pi]). ky2/ky4 = wrap(2ky), wrap(4ky) come from the host. All four
    1024-wide trig batches are emitted ahead of the MM loop so the Scalar
    engine streams Sin evaluations while the gather runs.
"""

import sys

if "/opt/trn_rl_repo" not in sys.path:
    sys.path.insert(0, "/opt/trn_rl_repo")

import math

import numpy as np
import ml_dtypes

import concourse.bass as bass
import concourse.tile as tile
from concourse import bacc
from concourse import mybir
from concourse import dve_ops
from concourse.dve_spec import Spec, Src0, Src1, C0, C1, C2, Zero, maxx

P = 128
NX = 128
NCOIL = 8
NS = 8192
NT = 4
S = 4096           # samples per core (half of NS)
CH = 512           # samples per MM chunk (PSUM bank width)
NCHUNK = S // CH   # 8
BW = 1024          # trig batch width (2 chunks)
NBATCH = S // BW   # 4
YI = 8
YO = 16
NPIX = NX * NX
NGATH = 16
GIDX = NPIX // NGATH   # 1024 indices per gather
ELEM = 128             # bf16 elements per table row = 256 bytes
NABS_SC = 4            # yi < NABS_SC: cos-prep via scalar Abs; else DVE ABS_SUB
GP_OUTER = 0           # gpsimd cannot read PSUM: outer products stay on DVE

F32 = mybir.dt.float32
BF16 = mybir.dt.bfloat16
I16 = mybir.dt.int16
TWO_PI = float(2.0 * math.pi)
MAGIC = 12582912.0  # 1.5*2^23: (x + M) - M == round-to-nearest(x) for f32
ALU = mybir.AluOpType
ACTF = mybir.ActivationFunctionType


# ---------------- custom DVE ops ----------------
def _register_dve_op(name, spec):
    if name in dve_ops._SUB_OPCODE_FOR_NAME:
        for op in dve_ops.OPS:
            if op.name == name:
                return op
        raise RuntimeError(name)
    shas = {}
    for ver in ("v3", "v4"):
        uops = dve_ops.lower(spec, ver=ver)
        shas[ver] = dve_ops.DveOpSpec(
            name=name, opcode=0, uops=uops, rd1_en=dve_ops.has_src1(spec)
        ).sha(ver)
    op = dve_ops.DveOp(name, spec, subdim=False, uops_sha=shas)
    dve_ops.OPS.append(op)
    dve_ops._SUB_OPCODE_FOR_NAME[name] = (
        dve_ops._CUSTOM_DVE_ROW_BASE + len(dve_ops.OPS) - 1
    )
    dve_ops.CUSTOM_DVE_SPECS[name] = spec
    return op


def _wrap_np(v):
    return (v - np.round(v)).astype(np.float32)


# out = m - round(m), m = in0*s0 + in1*s1  (s0 may be a [P,1] AP)
_pw_m = Src0 * C0 + Src1 * C1
_pw_r = (_pw_m + C2) - C2
PHASE_WRAP = _register_dve_op(
    "PHASE_WRAP_ANT",
    Spec(
        body=_pw_m - _pw_r,
        reference=lambda in0, in1, s0, s1, imm2: (
            (in0 * s0 + in1 * s1)
            - (((in0 * s0 + in1 * s1) + imm2) - imm2)
        ).astype(np.float32),
    ),
)

# out = y - ((y > .5) - (y < -.5)), y = in0 + in1 : one-period wrap of a sum
_aw_y = Src0 + Src1
ADD_TT_WRAP = _register_dve_op(
    "ADD_TT_WRAP_ANT",
    Spec(
        body=_aw_y + C2 * ((_aw_y < (Zero - C1)) - (C1 < _aw_y)),
        reference=lambda in0, in1, s0, s1, imm2: (
            (in0 + in1)
            + imm2
            * (
                ((in0 + in1) < -s1).astype(np.float32)
                - ((in0 + in1) > s1).astype(np.float32)
            )
        ).astype(np.float32),
    ),
)

# out = |in0| + s0
ABS_SUB = _register_dve_op(
    "ABS_ADD_ANT",
    Spec(
        body=maxx(Src0, Zero - Src0) + C0,
        reference=lambda in0, in1, s0, s1, imm2: (np.abs(in0) + s0).astype(
            np.float32
        ),
    ),
)


def build_program(nc: bass.Bass, dbg: bool = False):
    def dbg_out(name, src_ap, shape, dtype=F32):
        if not dbg:
            return
        d = nc.dram_tensor("dbg_" + name, shape, dtype, kind="ExternalOutput").ap()
        nc.sync.dma_start(d[:], src_ap)

    csm_r = nc.dram_tensor("csm_r", [NCOIL, NX, NX], F32, kind="ExternalInput").ap()
    csm_i = nc.dram_tensor("csm_i", [NCOIL, NX, NX], F32, kind="ExternalInput").ap()
    kx_d = nc.dram_tensor("kx", [S], F32, kind="ExternalInput").ap()
    ky_d = nc.dram_tensor("ky", [S], F32, kind="ExternalInput").ap()
    tbl_d = nc.dram_tensor("tbl", [NPIX, ELEM], BF16, kind="ExternalInput").ap()
    idx_d = nc.dram_tensor("idx", [P, GIDX], I16, kind="ExternalInput").ap()
    w4_d = nc.dram_tensor("w4", [P, NX, 4], F32, kind="ExternalInput").ap()
    out_r = nc.dram_tensor("out_r", [NCOIL, S], F32, kind="ExternalOutput").ap()
    out_i = nc.dram_tensor("out_i", [NCOIL, S], F32, kind="ExternalOutput").ap()

    # ---------------- inline constants ----------------
    pvals = np.arange(P, dtype=np.float32)
    xc_d = nc.inline_tensor((pvals - 64.0).reshape(P, 1), name="c_xc").ap()
    yo8_d = nc.inline_tensor((8.0 * (np.arange(P) % 16)).astype(np.float32)
                             .reshape(P, 1), name="c_yo8").ap()
    half_pi_d = nc.inline_tensor(np.full((P, 1), math.pi / 2, np.float32),
                                 name="c_half_pi").ap()
    sel_np = (np.arange(P)[:, None] // 16 == np.arange(NCOIL)[None, :]).astype(
        np.float32)
    selpm_np = np.concatenate([sel_np, -sel_np], axis=1)  # [128, 16]: +sel | -sel
    selpm_d = nc.inline_tensor(selpm_np, name="c_selpm").ap()

    with tile.TileContext(nc) as tc, \
         tc.tile_pool(name="pp", bufs=1) as pp:

        # --- persistent constants / inputs ---
        idx16 = pp.tile([P, GIDX], I16)
        H = S // 2
        kxb = pp.tile([P, S], F32)
        nc.scalar.dma_start(
            kxb[:, 0:H],
            kx_d[0:H].rearrange("(p s) -> p s", p=1).to_broadcast([P, H]))
        kyb = pp.tile([P, S], F32)
        nc.sync.dma_start(
            kyb[:, 0:BW],
            ky_d[0:BW].rearrange("(p s) -> p s", p=1).to_broadcast([P, BW]))
        xc_col = pp.tile([P, 1], F32)
        nc.sync.dma_start(xc_col[:], xc_d[:])
        yo8 = pp.tile([P, 1], F32)
        nc.sync.dma_start(yo8[:], yo8_d[:])
        half_pi = pp.tile([P, 1], F32)
        nc.sync.dma_start(half_pi[:], half_pi_d[:])
        selpm32 = pp.tile([P, 2 * NCOIL], F32)
        nc.sync.dma_start(selpm32[:], selpm_d[:])
        selpm = pp.tile([P, 2 * NCOIL], BF16)
        nc.vector.tensor_copy(selpm[:], selpm32[:])

        # packed coil stationary: blocks [Cr | Ci | -Ci], col = c*16 + yo,
        # innermost yi so the pack writes contiguous 16B runs
        RA = pp.tile([P, 3, P, YI], BF16)

        # --- pools (gp innermost so it can close after the warp) ---
        lp_ctx = tc.tile_pool(name="loop", bufs=1)
        lp = lp_ctx.__enter__()
        kp_ctx = tc.tile_pool(name="kr", bufs=1)
        kp = kp_ctx.__enter__()
        gp_pool_ctx = tc.tile_pool(name="gp", bufs=1)
        gp = gp_pool_ctx.__enter__()
        g8p = gp.tile([P, NX, ELEM], BF16)
        w4sb = gp.tile([P, NX, 4], F32)
        nc.scalar.dma_start(w4sb[:], w4_d[:])
        csm_r_sb = gp.tile([P, NCOIL, NX], F32)
        csm_i_sb = gp.tile([P, NCOIL, NX], F32)

        # idx16 is loaded LAST on the sync queue: HWDGE executes FIFO, so the
        # gathers (which depend on idx16) cannot start stealing SDMA slots
        # until every other input DMA has landed.
        nc.sync.dma_start(idx16[:], idx_d[:])
        nc.sync.dma_start(csm_r_sb[:], csm_r.rearrange("c x y -> x c y"))
        nc.sync.dma_start(csm_i_sb[:], csm_i.rearrange("c x y -> x c y"))
        nc.sync.dma_start(
            kyb[:, BW:S],
            ky_d[BW:S].rearrange("(p s) -> p s", p=1)
            .to_broadcast([P, S - BW]))
        nc.sync.dma_start(
            kxb[:, H:S],
            kx_d[H:S].rearrange("(p s) -> p s", p=1).to_broadcast([P, S - H]))
        nc.sync.dma_start(
            kyb[:, H:S],
            ky_d[H:S].rearrange("(p s) -> p s", p=1).to_broadcast([P, S - H]))
        gsems = [nc.alloc_semaphore(f"gath_sem{q}") for q in range(4)]
        for h in range(NGATH):
            nc.gpsimd.dma_gather(
                out_ap=g8p[:, h * 8:(h + 1) * 8, :],
                in_ap=tbl_d[:],
                idxs_ap=idx16[:, h * 64:(h + 1) * 64],
                num_idxs=GIDX,
                num_idxs_reg=GIDX,
                elem_size=ELEM,
                queue_num=h % 4,
            ).then_inc(gsems[h % 4], 16)

        # ---------------- trig batches (emitted in pieces) ----------------
        trig = {}

        def make_trig(b):
            cs = slice(b * BW, (b + 1) * BW)
            nabs = NABS_SC
            st = {"m2": {}, "kits": [], "krts": []}

            def emit_yi(yi):
                m2 = st["m2"]
                kyc = st["kyc"]
                if yi > 0:
                    t = lp.tile([P, BW], F32, tag="m2c", bufs=2)
                    nc.vector._custom_dve(ADD_TT_WRAP, out=t[:],
                                          in0=m2[yi - 1][:],
                                          in1=kyc, s1=0.5, imm2=1.0)
                    m2[yi] = t
                kit = kp.tile([P, BW], BF16, tag=f"kit{yi}", bufs=2)
                nc.scalar.activation(kit[:], m2[yi][:], ACTF.Sin, scale=-TWO_PI)
                krt = kp.tile([P, BW], BF16, tag=f"krt{yi}", bufs=2)
                if yi < nabs:
                    mabs = lp.tile([P, BW], F32, tag="mabs", bufs=1)
                    nc.scalar.activation(mabs[:], m2[yi][:], ACTF.Abs)
                    nc.scalar.activation(krt[:], mabs[:], ACTF.Sin,
                                         scale=-TWO_PI, bias=half_pi[:, 0:1])
                else:
                    mk = lp.tile([P, BW], F32, tag="mk", bufs=2)
                    nc.vector._custom_dve(ABS_SUB, out=mk[:], in0=m2[yi][:],
                                          s0=-0.25)
                    nc.scalar.activation(krt[:], mk[:], ACTF.Sin, scale=-TWO_PI)
                st["kits"].append(kit)
                st["krts"].append(krt)

            def piece0():
                kxc = kxb[:, cs]
                kyc = st["kyc"] = kyb[:, cs]
                m2o = lp.tile([P, BW], F32, tag="m2o", bufs=1)
                nc.vector._custom_dve(PHASE_WRAP, out=m2o[:], in0=kyc,
                                      in1=kyc, s0=yo8[:, 0:1], s1=0.0,
                                      imm2=MAGIC)
                mok = lp.tile([P, BW], F32, tag="mok", bufs=1)
                nc.vector._custom_dve(ABS_SUB, out=mok[:], in0=m2o[:], s0=-0.25)
                aic = kp.tile([P, BW], BF16, tag="aic", bufs=2)
                nc.scalar.activation(aic[:], m2o[:], ACTF.Sin, scale=-TWO_PI)
                arc = kp.tile([P, BW], BF16, tag="arc", bufs=2)
                nc.scalar.activation(arc[:], mok[:], ACTF.Sin, scale=-TWO_PI)
                m2a = lp.tile([P, BW], F32, tag="m2a", bufs=1)
                nc.vector._custom_dve(PHASE_WRAP, out=m2a[:], in0=kxc,
                                      in1=kyc, s0=xc_col[:, 0:1], s1=-64.0,
                                      imm2=MAGIC)
                st["m2"][0] = m2a
                emit_yi(0)
                trig[b] = (st["kits"], st["krts"], arc, aic)

            return [piece0] + [lambda yi=yi: emit_yi(yi) for yi in range(1, YI)]

        # ---------------- warp eighth: combine + pack ----------------
        def emit_quarter(q):
            W = 16
            ys = slice(W * q, W * q + W)
            for h in (2 * q, 2 * q + 1):
                nc.vector.wait_ge(gsems[h % 4], 16 * (h // 4 + 1))
            t8r = gp.tile([P, W, 4], F32, tag="t8r", bufs=2)
            nc.vector.tensor_tensor(t8r[:], g8p[:, ys, 0:4], w4sb[:, ys, :],
                                    op=ALU.mult)
            warped_r = gp.tile([P, W], F32, tag="wr", bufs=2)
            nc.vector.reduce_sum(warped_r[:], t8r[:], axis=mybir.AxisListType.X)
            t8i = gp.tile([P, W, 4], F32, tag="t8i", bufs=2)
            nc.vector.tensor_tensor(t8i[:], g8p[:, ys, 4:8], w4sb[:, ys, :],
                                    op=ALU.mult)
            warped_i = gp.tile([P, W], F32, tag="wi", bufs=2)
            nc.vector.reduce_sum(warped_i[:], t8i[:], axis=mybir.AxisListType.X)

            wr_b = warped_r[:].rearrange("p (c y) -> p c y", c=1).to_broadcast(
                [P, NCOIL, W])
            wi_b = warped_i[:].rearrange("p (c y) -> p c y", c=1).to_broadcast(
                [P, NCOIL, W])
            csr = csm_r_sb[:, :, ys]
            csi = csm_i_sb[:, :, ys]

            # RA views for this eighth: [p, c, yo(2), yi(8)], contiguous yi
            ra5 = RA[:].rearrange("p b (c yo) yi -> p b c yo yi", c=NCOIL)
            NYO = W // YI
            cr_v = ra5[:, 0, :, NYO * q:NYO * q + NYO, :]
            ci_v = ra5[:, 1, :, NYO * q:NYO * q + NYO, :]
            cin_v = ra5[:, 2, :, NYO * q:NYO * q + NYO, :]

            def as4(t):
                return t.rearrange("p c (yo yi) -> p c yo yi", yi=YI)

            tt1 = gp.tile([P, NCOIL, W], F32, tag="tt1", bufs=2)
            nc.vector.tensor_tensor(tt1[:], csr, wr_b, op=ALU.mult)
            tt2 = gp.tile([P, NCOIL, W], F32, tag="tt2", bufs=2)
            nc.vector.tensor_tensor(tt2[:], csi, wi_b, op=ALU.mult)
            nc.vector.tensor_tensor(cr_v, as4(tt1[:]), as4(tt2[:]),
                                    op=ALU.subtract)
            tt3 = gp.tile([P, NCOIL, W], F32, tag="tt1", bufs=2)
            nc.vector.tensor_tensor(tt3[:], csr, wi_b, op=ALU.mult)
            tt4 = gp.tile([P, NCOIL, W], F32, tag="tt2", bufs=2)
            nc.vector.tensor_tensor(tt4[:], csi, wr_b, op=ALU.mult)
            cit = gp.tile([P, NCOIL, W], F32, tag="cit", bufs=2)
            nc.vector.tensor_tensor(cit[:], tt3[:], tt4[:], op=ALU.add)
            nc.vector.tensor_copy(ci_v, as4(cit[:]))
            nc.vector.tensor_scalar(cin_v, as4(cit[:]), -1.0, None, op0=ALU.mult)

        # ---------------- MM chunk ----------------
        ps_ctx = tc.tile_pool(name="ps", bufs=1, space="PSUM")
        ps = ps_ctx.__enter__()
        pso_ctx = tc.tile_pool(name="pso", bufs=1, space="PSUM")
        pso = pso_ctx.__enter__()

        live = {}

        def emit_mains(ch):
            b, half = divmod(ch, 2)
            sl = slice(half * CH, (half + 1) * CH)
            kits, krts, arc, aic = trig[b]
            Pr = ps.tile([P, CH], F32, tag="Pr", bufs=3)
            Pi = ps.tile([P, CH], F32, tag="Pi", bufs=3)
            for yi in range(YI):
                st, sp = (yi == 0), (yi == YI - 1)
                krt_s = krts[yi][:, sl]
                kit_s = kits[yi][:, sl]
                nc.tensor.matmul(Pr[:], RA[:, 0, :, yi], krt_s,
                                 start=st, stop=False)
                nc.tensor.matmul(Pi[:], RA[:, 0, :, yi], kit_s,
                                 start=st, stop=False)
                nc.tensor.matmul(Pr[:], RA[:, 2, :, yi], kit_s,
                                 start=False, stop=sp)
                nc.tensor.matmul(Pi[:], RA[:, 1, :, yi], krt_s,
                                 start=False, stop=sp)
            live[ch] = (Pr, Pi, arc, aic, sl)

        def emit_post(ch):
            c0 = ch * CH
            Pr, Pi, arc, aic, sl = live.pop(ch)
            q1 = lp.tile([P, CH], BF16, tag="q1", bufs=2)
            nc.vector.tensor_tensor(q1[:], Pr[:], arc[:, sl], op=ALU.mult)
            q2 = lp.tile([P, CH], BF16, tag="q2", bufs=2)
            nc.vector.tensor_tensor(q2[:], Pi[:], aic[:, sl], op=ALU.mult)
            eng3 = nc.gpsimd if GP_OUTER >= 1 else nc.vector
            eng4 = nc.gpsimd if GP_OUTER >= 2 else nc.vector
            q3 = lp.tile([P, CH], BF16, tag="q3", bufs=2)
            eng3.tensor_tensor(q3[:], Pi[:], arc[:, sl], op=ALU.mult)
            q4 = lp.tile([P, CH], BF16, tag="q4", bufs=2)
            eng4.tensor_tensor(q4[:], Pr[:], aic[:, sl], op=ALU.mult)

            SP, SM = selpm[:, 0:NCOIL], selpm[:, NCOIL:2 * NCOIL]
            po = pso.tile([32 + NCOIL, CH], F32, tag="po", bufs=2)
            nc.tensor.matmul(po[0:NCOIL], SP, q1[:], start=True, stop=False)
            nc.tensor.matmul(po[0:NCOIL], SM, q2[:], start=False, stop=True)
            nc.tensor.matmul(po[32:32 + NCOIL], SP, q3[:], start=True,
                             stop=False)
            nc.tensor.matmul(po[32:32 + NCOIL], SP, q4[:], start=False,
                             stop=True)
            ost = lp.tile([32 + NCOIL, CH], F32, tag="ost", bufs=2)
            nc.scalar.copy(ost[:], po[:])
            nc.sync.dma_start(out_r[:, c0:c0 + CH], ost[0:NCOIL])
            nc.sync.dma_start(out_i[:, c0:c0 + CH], ost[32:32 + NCOIL])

        # ---------------- emission schedule ----------------
        for p in make_trig(0):
            p()
        for q in range(8):
            emit_quarter(q)
        dbg_out("RA", RA[:].rearrange("p b c yi -> p (b c yi)"), [P, YI * 3 * P],
                BF16)
        gp_pool_ctx.__exit__(None, None, None)
        for p in make_trig(1):
            p()

        # interleave trig batches 2/3 into the MM loop so the DVE queue never
        # blocks the selector matmuls: b2 pieces land after posts 1-3, b3
        # after posts 3-5.
        t2 = make_trig(2)
        t3 = make_trig(3)
        pieces = {0: t2[0:3], 1: t2[3:6], 2: t2[6:8] + t3[0:1],
                  3: t3[1:4], 4: t3[4:8]}

        for ch in range(NCHUNK):
            emit_mains(ch)
            if ch > 0:
                emit_post(ch - 1)
                for p in pieces.get(ch - 1, []):
                    p()
        emit_post(NCHUNK - 1)

        pso_ctx.__exit__(None, None, None)
        ps_ctx.__exit__(None, None, None)
        kp_ctx.__exit__(None, None, None)
        lp_ctx.__exit__(None, None, None)


_COMPILED = {}


def _get_nc(dbg: bool = False):
    key = ("nc", dbg)
    if key not in _COMPILED:
        nc = bacc.Bacc("TRN2", debug=False, num_swdge_queues=4)
        build_program(nc, dbg=dbg)
        nc.compile()
        _COMPILED[key] = nc
    return _COMPILED[key]


# slot g = 16*j + (p%16)  <->  output pixel (x = g%128, y = g//128);
# gather h covers slots [1024h, 1024(h+1)) -> partitions x, columns y.
_Jg = np.arange(GIDX)[None, :]
_Pg = np.arange(P)[:, None]
_G = 16 * _Jg + (_Pg % 16)            # [128, 1024]
_XG = (_G % 128).astype(np.int64)
_YG = (_G // 128).astype(np.int64)
_BF16 = ml_dtypes.bfloat16


def _build_tables(image_r, image_i, flow):
    """Per-timepoint: corner table (bf16, row y0*128+x0), idx16, weights."""
    ir = np.ascontiguousarray(image_r, np.float32)
    ii = np.ascontiguousarray(image_i, np.float32)
    irT, iiT = ir.T, ii.T                     # [y, x]
    y1 = np.minimum(np.arange(NX) + 1, NX - 1)
    x1 = np.minimum(np.arange(NX) + 1, NX - 1)
    tables = []
    for t in range(NT):
        f0 = np.asarray(flow[:, :, 0, t], np.float32)
        f1 = np.asarray(flow[:, :, 1, t], np.float32)
        # float32 math mirrors the jax reference exactly
        xg = np.arange(NX, dtype=np.float32)[:, None]
        yg = np.arange(NX, dtype=np.float32)[None, :]
        cx = np.clip(xg + f0, np.float32(0.0), np.float32(NX - 1))
        cy = np.clip(yg + f1, np.float32(0.0), np.float32(NX - 1))
        x0 = np.floor(cx)
        y0 = np.floor(cy)
        wx = (cx - x0).astype(np.float32)     # [x, y]
        wy = (cy - y0).astype(np.float32)
        w4 = np.stack([(1 - wx) * (1 - wy), (1 - wx) * wy,
                       wx * (1 - wy), wx * wy], axis=-1).astype(np.float32)
        x0i = x0.astype(np.int64)
        y0i = y0.astype(np.int64)
        idxv = (y0i * NX + x0i).astype(np.int16)      # [x, y]
        idx16 = idxv[_XG, _YG]                        # wrapped gather layout

        tbl = np.zeros((NX, NX, ELEM), dtype=_BF16)
        tbl[:, :, 0] = irT
        tbl[:, :, 1] = irT[y1, :]
        tbl[:, :, 2] = irT[:, x1]
        tbl[:, :, 3] = irT[y1][:, x1]
        tbl[:, :, 4] = iiT
        tbl[:, :, 5] = iiT[y1, :]
        tbl[:, :, 6] = iiT[:, x1]
        tbl[:, :, 7] = iiT[y1][:, x1]
        tables.append((tbl.reshape(NPIX, ELEM), idx16, w4))
    return tables


def make_in_maps(image_r, image_i, csm_r, csm_i, traj, dcf, flow):
    del dcf  # unused by the operator
    tables = _build_tables(image_r, image_i, flow)
    csm_r = np.ascontiguousarray(csm_r, np.float32)
    csm_i = np.ascontiguousarray(csm_i, np.float32)
    in_maps = []
    for core in range(8):
        t, h = divmod(core, 2)
        sl = slice(h * S, (h + 1) * S)
        tbl, idx16, w4 = tables[t]
        in_maps.append({
            "csm_r": csm_r,
            "csm_i": csm_i,
            "kx": np.ascontiguousarray(traj[sl, 0, t], np.float32),
            "ky": np.ascontiguousarray(traj[sl, 1, t], np.float32),
            "tbl": np.ascontiguousarray(tbl),
            "idx": np.ascontiguousarray(idx16),
            "w4": np.ascontiguousarray(w4),
        })
    return in_maps


def combine_outputs(results):
    out = np.zeros((NCOIL, NS), np.complex64)
    for core, res in enumerate(results):
        t, h = divmod(core, 2)
        sl = slice(h * S, (h + 1) * S)
        out[:, sl] += res["out_r"].astype(np.complex64) + 1j * res["out_i"].astype(
            np.complex64)
    return out


def kernel(**inputs) -> np.ndarray:
    from concourse.bass_utils import run_bass_kernel_spmd

    nc = _get_nc()
    in_maps = make_in_maps(**inputs)
    res = run_bass_kernel_spmd(nc, in_maps, core_ids=list(range(8)))
    return combine_outputs(res.results)


# revision 26
# speedup vs baseline: 1.0155x; 1.0155x over previous
"""Batchelor GPU-NUFFT forward operator on 8 Trainium2 NeuronCores.

Math (per timepoint t):
    warped  = bilinear_warp(image, flow[..., t])
    coil    = csm * warped                                  [Nc,Nx,Ny]
    out_t[c,s] = sum_{x,y} coil[c,x,y] exp(-2pi i (kx_s (x-64) + ky_s (y-64)))
    out     = sum_t out_t                                   [Nc,NS] complex64

Sharding: 8 cores = 4 timepoints x 2 sample-halves (4096 samples each).
Host unshard: sum the 4 timepoint partials per half, concat halves.

Device pipeline (per core):
  * warp: host provides the bf16 corner table (DRAM, row (y0*128+x0) holds the
    4 bilinear corners of real+imag), int16 gather indices in the SWDGE
    wrapped layout, and the 4 bilinear weight planes. 16 dma_gather ops land
    the corners directly in [x, y] layout (slot i = y*128 + x); the combine
    and the coil pack run per 4-gather quarter in the gather shadow.
  * NUFFT: Khatri-Rao split y = yo*8 + yi. Per 512-sample chunk, 32
    accumulating bf16 matmuls build PSUM partials Pr = Re(sum coil e^{-iA}),
    Pi = Im(...) directly (stationary blocks Cr | Ci | -Ci make the +- signs
    accumulate in PSUM). The outer phase e^{-iB} is 4 elementwise products,
    folded to 8 coils by +-selector matmuls.
  * trig: phases are range-reduced with custom DVE ops (PHASE_WRAP fuses the
    a0 = kx*(x-64) - 64*ky wrap to one op; ADD_TT_WRAP fuses each chain step
    m2_yi = wrap(m2_parent + ky2^j) with a log-depth parent tree; ABS_SUB
    preps cos args as |m|-1/4 since the ACT Sin spline is only valid on
    [-pi, pi]). ky2/ky4 = wrap(2ky), wrap(4ky) come from the host. All four
    1024-wide trig batches are emitted ahead of the MM loop so the Scalar
    engine streams Sin evaluations while the gather runs.
"""

import sys

if "/opt/trn_rl_repo" not in sys.path:
    sys.path.insert(0, "/opt/trn_rl_repo")

import math

import numpy as np
import ml_dtypes

import concourse.bass as bass
import concourse.tile as tile
from concourse import bacc
from concourse import mybir
from concourse import dve_ops
from concourse.dve_spec import Spec, Src0, Src1, C0, C1, C2, Zero, maxx

P = 128
NX = 128
NCOIL = 8
NS = 8192
NT = 4
S = 4096           # samples per core (half of NS)
CH = 512           # samples per MM chunk (PSUM bank width)
NCHUNK = S // CH   # 8
BW = 1024          # trig batch width (2 chunks)
NBATCH = S // BW   # 4
YI = 8
YO = 16
NPIX = NX * NX
NGATH = 16
GIDX = NPIX // NGATH   # 1024 indices per gather
ELEM = 128             # bf16 elements per table row = 256 bytes
NABS_SC = 4            # yi < NABS_SC: cos-prep via scalar Abs; else DVE ABS_SUB
GP_OUTER = 0           # gpsimd cannot read PSUM: outer products stay on DVE

F32 = mybir.dt.float32
BF16 = mybir.dt.bfloat16
I16 = mybir.dt.int16
TWO_PI = float(2.0 * math.pi)
MAGIC = 12582912.0  # 1.5*2^23: (x + M) - M == round-to-nearest(x) for f32
ALU = mybir.AluOpType
ACTF = mybir.ActivationFunctionType


# ---------------- custom DVE ops ----------------
def _register_dve_op(name, spec):
    if name in dve_ops._SUB_OPCODE_FOR_NAME:
        for op in dve_ops.OPS:
            if op.name == name:
                return op
        raise RuntimeError(name)
    shas = {}
    for ver in ("v3", "v4"):
        uops = dve_ops.lower(spec, ver=ver)
        shas[ver] = dve_ops.DveOpSpec(
            name=name, opcode=0, uops=uops, rd1_en=dve_ops.has_src1(spec)
        ).sha(ver)
    op = dve_ops.DveOp(name, spec, subdim=False, uops_sha=shas)
    dve_ops.OPS.append(op)
    dve_ops._SUB_OPCODE_FOR_NAME[name] = (
        dve_ops._CUSTOM_DVE_ROW_BASE + len(dve_ops.OPS) - 1
    )
    dve_ops.CUSTOM_DVE_SPECS[name] = spec
    return op


def _wrap_np(v):
    return (v - np.round(v)).astype(np.float32)


# out = m - round(m), m = in0*s0 + in1*s1  (s0 may be a [P,1] AP)
_pw_m = Src0 * C0 + Src1 * C1
_pw_r = (_pw_m + C2) - C2
PHASE_WRAP = _register_dve_op(
    "PHASE_WRAP_ANT",
    Spec(
        body=_pw_m - _pw_r,
        reference=lambda in0, in1, s0, s1, imm2: (
            (in0 * s0 + in1 * s1)
            - (((in0 * s0 + in1 * s1) + imm2) - imm2)
        ).astype(np.float32),
    ),
)

# out = y - ((y > .5) - (y < -.5)), y = in0 + in1 : one-period wrap of a sum
_aw_y = Src0 + Src1
ADD_TT_WRAP = _register_dve_op(
    "ADD_TT_WRAP_ANT",
    Spec(
        body=_aw_y + C2 * ((_aw_y < (Zero - C1)) - (C1 < _aw_y)),
        reference=lambda in0, in1, s0, s1, imm2: (
            (in0 + in1)
            + imm2
            * (
                ((in0 + in1) < -s1).astype(np.float32)
                - ((in0 + in1) > s1).astype(np.float32)
            )
        ).astype(np.float32),
    ),
)

# out = |in0| + s0
ABS_SUB = _register_dve_op(
    "ABS_ADD_ANT",
    Spec(
        body=maxx(Src0, Zero - Src0) + C0,
        reference=lambda in0, in1, s0, s1, imm2: (np.abs(in0) + s0).astype(
            np.float32
        ),
    ),
)


def build_program(nc: bass.Bass, dbg: bool = False):
    def dbg_out(name, src_ap, shape, dtype=F32):
        if not dbg:
            return
        d = nc.dram_tensor("dbg_" + name, shape, dtype, kind="ExternalOutput").ap()
        nc.sync.dma_start(d[:], src_ap)

    csm_r = nc.dram_tensor("csm_r", [NCOIL, NX, NX], F32, kind="ExternalInput").ap()
    csm_i = nc.dram_tensor("csm_i", [NCOIL, NX, NX], F32, kind="ExternalInput").ap()
    kx_d = nc.dram_tensor("kx", [S], F32, kind="ExternalInput").ap()
    ky_d = nc.dram_tensor("ky", [S], F32, kind="ExternalInput").ap()
    tbl_d = nc.dram_tensor("tbl", [NPIX, ELEM], BF16, kind="ExternalInput").ap()
    idx_d = nc.dram_tensor("idx", [P, GIDX], I16, kind="ExternalInput").ap()
    w4_d = nc.dram_tensor("w4", [P, NX, 4], F32, kind="ExternalInput").ap()
    out_r = nc.dram_tensor("out_r", [NCOIL, S], F32, kind="ExternalOutput").ap()
    out_i = nc.dram_tensor("out_i", [NCOIL, S], F32, kind="ExternalOutput").ap()

    # ---------------- inline constants ----------------
    pvals = np.arange(P, dtype=np.float32)
    xc_d = nc.inline_tensor((pvals - 64.0).reshape(P, 1), name="c_xc").ap()
    yo8_d = nc.inline_tensor((8.0 * (np.arange(P) % 16)).astype(np.float32)
                             .reshape(P, 1), name="c_yo8").ap()
    half_pi_d = nc.inline_tensor(np.full((P, 1), math.pi / 2, np.float32),
                                 name="c_half_pi").ap()
    sel_np = (np.arange(P)[:, None] // 16 == np.arange(NCOIL)[None, :]).astype(
        np.float32)
    selpm_np = np.concatenate([sel_np, -sel_np], axis=1)  # [128, 16]: +sel | -sel
    selpm_d = nc.inline_tensor(selpm_np, name="c_selpm").ap()

    with tile.TileContext(nc) as tc, \
         tc.tile_pool(name="pp", bufs=1) as pp:

        # --- persistent constants / inputs ---
        idx16 = pp.tile([P, GIDX], I16)
        H = S // 2
        kxb = pp.tile([P, S], F32)
        nc.scalar.dma_start(
            kxb[:, 0:H],
            kx_d[0:H].rearrange("(p s) -> p s", p=1).to_broadcast([P, H]))
        kyb = pp.tile([P, S], F32)
        nc.sync.dma_start(
            kyb[:, 0:H],
            ky_d[0:H].rearrange("(p s) -> p s", p=1).to_broadcast([P, H]))
        xc_col = pp.tile([P, 1], F32)
        nc.sync.dma_start(xc_col[:], xc_d[:])
        yo8 = pp.tile([P, 1], F32)
        nc.sync.dma_start(yo8[:], yo8_d[:])
        half_pi = pp.tile([P, 1], F32)
        nc.sync.dma_start(half_pi[:], half_pi_d[:])
        selpm32 = pp.tile([P, 2 * NCOIL], F32)
        nc.sync.dma_start(selpm32[:], selpm_d[:])
        selpm = pp.tile([P, 2 * NCOIL], BF16)
        nc.vector.tensor_copy(selpm[:], selpm32[:])

        # packed coil stationary: blocks [Cr | Ci | -Ci], col = c*16 + yo,
        # innermost yi so the pack writes contiguous 16B runs
        RA = pp.tile([P, 3, P, YI], BF16)

        # --- pools (gp innermost so it can close after the warp) ---
        lp_ctx = tc.tile_pool(name="loop", bufs=1)
        lp = lp_ctx.__enter__()
        kp_ctx = tc.tile_pool(name="kr", bufs=1)
        kp = kp_ctx.__enter__()
        gp_pool_ctx = tc.tile_pool(name="gp", bufs=1)
        gp = gp_pool_ctx.__enter__()
        g8p = gp.tile([P, NX, ELEM], BF16)
        w4sb = gp.tile([P, NX, 4], F32)
        nc.scalar.dma_start(w4sb[:], w4_d[:])
        csm_r_sb = gp.tile([P, NCOIL, NX], F32)
        nc.sync.dma_start(csm_r_sb[:], csm_r.rearrange("c x y -> x c y"))
        csm_i_sb = gp.tile([P, NCOIL, NX], F32)
        nc.sync.dma_start(csm_i_sb[:], csm_i.rearrange("c x y -> x c y"))

        # idx16 is loaded LAST on the sync queue: HWDGE executes FIFO, so the
        # gathers (which depend on idx16) cannot start stealing SDMA slots
        # until every other input DMA has landed.
        nc.sync.dma_start(idx16[:], idx_d[:])
        nc.sync.dma_start(
            kxb[:, H:S],
            kx_d[H:S].rearrange("(p s) -> p s", p=1).to_broadcast([P, S - H]))
        nc.sync.dma_start(
            kyb[:, H:S],
            ky_d[H:S].rearrange("(p s) -> p s", p=1).to_broadcast([P, S - H]))
        gsems = [nc.alloc_semaphore(f"gath_sem{q}") for q in range(4)]
        for h in range(NGATH):
            nc.gpsimd.dma_gather(
                out_ap=g8p[:, h * 8:(h + 1) * 8, :],
                in_ap=tbl_d[:],
                idxs_ap=idx16[:, h * 64:(h + 1) * 64],
                num_idxs=GIDX,
                num_idxs_reg=GIDX,
                elem_size=ELEM,
                queue_num=h % 4,
            ).then_inc(gsems[h % 4], 16)

        # ---------------- trig batches (emitted in pieces) ----------------
        trig = {}

        def make_trig(b):
            cs = slice(b * BW, (b + 1) * BW)
            nabs = NABS_SC
            st = {"m2": {}, "kits": [], "krts": []}

            def emit_yi(yi):
                m2 = st["m2"]
                kyc = st["kyc"]
                if yi > 0:
                    t = lp.tile([P, BW], F32, tag="m2c", bufs=2)
                    nc.vector._custom_dve(ADD_TT_WRAP, out=t[:],
                                          in0=m2[yi - 1][:],
                                          in1=kyc, s1=0.5, imm2=1.0)
                    m2[yi] = t
                kit = kp.tile([P, BW], BF16, tag=f"kit{yi}", bufs=2)
                nc.scalar.activation(kit[:], m2[yi][:], ACTF.Sin, scale=-TWO_PI)
                krt = kp.tile([P, BW], BF16, tag=f"krt{yi}", bufs=2)
                if yi < nabs:
                    mabs = lp.tile([P, BW], F32, tag="mabs", bufs=1)
                    nc.scalar.activation(mabs[:], m2[yi][:], ACTF.Abs)
                    nc.scalar.activation(krt[:], mabs[:], ACTF.Sin,
                                         scale=-TWO_PI, bias=half_pi[:, 0:1])
                else:
                    mk = lp.tile([P, BW], F32, tag="mk", bufs=2)
                    nc.vector._custom_dve(ABS_SUB, out=mk[:], in0=m2[yi][:],
                                          s0=-0.25)
                    nc.scalar.activation(krt[:], mk[:], ACTF.Sin, scale=-TWO_PI)
                st["kits"].append(kit)
                st["krts"].append(krt)

            def piece0():
                kxc = kxb[:, cs]
                kyc = st["kyc"] = kyb[:, cs]
                m2o = lp.tile([P, BW], F32, tag="m2o", bufs=1)
                nc.vector._custom_dve(PHASE_WRAP, out=m2o[:], in0=kyc,
                                      in1=kyc, s0=yo8[:, 0:1], s1=0.0,
                                      imm2=MAGIC)
                mok = lp.tile([P, BW], F32, tag="mok", bufs=1)
                nc.vector._custom_dve(ABS_SUB, out=mok[:], in0=m2o[:], s0=-0.25)
                aic = kp.tile([P, BW], BF16, tag="aic", bufs=2)
                nc.scalar.activation(aic[:], m2o[:], ACTF.Sin, scale=-TWO_PI)
                arc = kp.tile([P, BW], BF16, tag="arc", bufs=2)
                nc.scalar.activation(arc[:], mok[:], ACTF.Sin, scale=-TWO_PI)
                m2a = lp.tile([P, BW], F32, tag="m2a", bufs=1)
                nc.vector._custom_dve(PHASE_WRAP, out=m2a[:], in0=kxc,
                                      in1=kyc, s0=xc_col[:, 0:1], s1=-64.0,
                                      imm2=MAGIC)
                st["m2"][0] = m2a
                emit_yi(0)
                trig[b] = (st["kits"], st["krts"], arc, aic)

            return [piece0] + [lambda yi=yi: emit_yi(yi) for yi in range(1, YI)]

        # ---------------- warp eighth: combine + pack ----------------
        def emit_quarter(q):
            W = 16
            ys = slice(W * q, W * q + W)
            for h in (2 * q, 2 * q + 1):
                nc.vector.wait_ge(gsems[h % 4], 16 * (h // 4 + 1))
            t8r = gp.tile([P, W, 4], F32, tag="t8r", bufs=2)
            nc.vector.tensor_tensor(t8r[:], g8p[:, ys, 0:4], w4sb[:, ys, :],
                                    op=ALU.mult)
            warped_r = gp.tile([P, W], F32, tag="wr", bufs=2)
            nc.vector.reduce_sum(warped_r[:], t8r[:], axis=mybir.AxisListType.X)
            t8i = gp.tile([P, W, 4], F32, tag="t8i", bufs=2)
            nc.vector.tensor_tensor(t8i[:], g8p[:, ys, 4:8], w4sb[:, ys, :],
                                    op=ALU.mult)
            warped_i = gp.tile([P, W], F32, tag="wi", bufs=2)
            nc.vector.reduce_sum(warped_i[:], t8i[:], axis=mybir.AxisListType.X)

            wr_b = warped_r[:].rearrange("p (c y) -> p c y", c=1).to_broadcast(
                [P, NCOIL, W])
            wi_b = warped_i[:].rearrange("p (c y) -> p c y", c=1).to_broadcast(
                [P, NCOIL, W])
            csr = csm_r_sb[:, :, ys]
            csi = csm_i_sb[:, :, ys]

            # RA views for this eighth: [p, c, yo(2), yi(8)], contiguous yi
            ra5 = RA[:].rearrange("p b (c yo) yi -> p b c yo yi", c=NCOIL)
            NYO = W // YI
            cr_v = ra5[:, 0, :, NYO * q:NYO * q + NYO, :]
            ci_v = ra5[:, 1, :, NYO * q:NYO * q + NYO, :]
            cin_v = ra5[:, 2, :, NYO * q:NYO * q + NYO, :]

            def as4(t):
                return t.rearrange("p c (yo yi) -> p c yo yi", yi=YI)

            tt1 = gp.tile([P, NCOIL, W], F32, tag="tt1", bufs=2)
            nc.vector.tensor_tensor(tt1[:], csr, wr_b, op=ALU.mult)
            tt2 = gp.tile([P, NCOIL, W], F32, tag="tt2", bufs=2)
            nc.vector.tensor_tensor(tt2[:], csi, wi_b, op=ALU.mult)
            nc.vector.tensor_tensor(cr_v, as4(tt1[:]), as4(tt2[:]),
                                    op=ALU.subtract)
            tt3 = gp.tile([P, NCOIL, W], F32, tag="tt1", bufs=2)
            nc.vector.tensor_tensor(tt3[:], csr, wi_b, op=ALU.mult)
            tt4 = gp.tile([P, NCOIL, W], F32, tag="tt2", bufs=2)
            nc.vector.tensor_tensor(tt4[:], csi, wr_b, op=ALU.mult)
            cit = gp.tile([P, NCOIL, W], F32, tag="cit", bufs=2)
            nc.vector.tensor_tensor(cit[:], tt3[:], tt4[:], op=ALU.add)
            nc.vector.tensor_copy(ci_v, as4(cit[:]))
            nc.vector.tensor_scalar(cin_v, as4(cit[:]), -1.0, None, op0=ALU.mult)

        # ---------------- MM chunk ----------------
        ps_ctx = tc.tile_pool(name="ps", bufs=1, space="PSUM")
        ps = ps_ctx.__enter__()
        pso_ctx = tc.tile_pool(name="pso", bufs=1, space="PSUM")
        pso = pso_ctx.__enter__()

        live = {}

        def emit_mains(ch):
            b, half = divmod(ch, 2)
            sl = slice(half * CH, (half + 1) * CH)
            kits, krts, arc, aic = trig[b]
            Pr = ps.tile([P, CH], F32, tag="Pr", bufs=3)
            Pi = ps.tile([P, CH], F32, tag="Pi", bufs=3)
            for yi in range(YI):
                st, sp = (yi == 0), (yi == YI - 1)
                krt_s = krts[yi][:, sl]
                kit_s = kits[yi][:, sl]
                nc.tensor.matmul(Pr[:], RA[:, 0, :, yi], krt_s,
                                 start=st, stop=False)
                nc.tensor.matmul(Pi[:], RA[:, 0, :, yi], kit_s,
                                 start=st, stop=False)
                nc.tensor.matmul(Pr[:], RA[:, 2, :, yi], kit_s,
                                 start=False, stop=sp)
                nc.tensor.matmul(Pi[:], RA[:, 1, :, yi], krt_s,
                                 start=False, stop=sp)
            live[ch] = (Pr, Pi, arc, aic, sl)

        def emit_post(ch):
            c0 = ch * CH
            Pr, Pi, arc, aic, sl = live.pop(ch)
            q1 = lp.tile([P, CH], BF16, tag="q1", bufs=2)
            nc.vector.tensor_tensor(q1[:], Pr[:], arc[:, sl], op=ALU.mult)
            q2 = lp.tile([P, CH], BF16, tag="q2", bufs=2)
            nc.vector.tensor_tensor(q2[:], Pi[:], aic[:, sl], op=ALU.mult)
            eng3 = nc.gpsimd if GP_OUTER >= 1 else nc.vector
            eng4 = nc.gpsimd if GP_OUTER >= 2 else nc.vector
            q3 = lp.tile([P, CH], BF16, tag="q3", bufs=2)
            eng3.tensor_tensor(q3[:], Pi[:], arc[:, sl], op=ALU.mult)
            q4 = lp.tile([P, CH], BF16, tag="q4", bufs=2)
            eng4.tensor_tensor(q4[:], Pr[:], aic[:, sl], op=ALU.mult)

            SP, SM = selpm[:, 0:NCOIL], selpm[:, NCOIL:2 * NCOIL]
            po = pso.tile([32 + NCOIL, CH], F32, tag="po", bufs=2)
            nc.tensor.matmul(po[0:NCOIL], SP, q1[:], start=True, stop=False)
            nc.tensor.matmul(po[0:NCOIL], SM, q2[:], start=False, stop=True)
            nc.tensor.matmul(po[32:32 + NCOIL], SP, q3[:], start=True,
                             stop=False)
            nc.tensor.matmul(po[32:32 + NCOIL], SP, q4[:], start=False,
                             stop=True)
            ost = lp.tile([32 + NCOIL, CH], F32, tag="ost", bufs=2)
            nc.scalar.copy(ost[:], po[:])
            nc.sync.dma_start(out_r[:, c0:c0 + CH], ost[0:NCOIL])
            nc.sync.dma_start(out_i[:, c0:c0 + CH], ost[32:32 + NCOIL])

        # ---------------- emission schedule ----------------
        for p in make_trig(0):
            p()
        for q in range(8):
            emit_quarter(q)
        dbg_out("RA", RA[:].rearrange("p b c yi -> p (b c yi)"), [P, YI * 3 * P],
                BF16)
        gp_pool_ctx.__exit__(None, None, None)
        for p in make_trig(1):
            p()

        # interleave trig batches 2/3 into the MM loop so the DVE queue never
        # blocks the selector matmuls: b2 pieces land after posts 1-3, b3
        # after posts 3-5.
        t2 = make_trig(2)
        t3 = make_trig(3)
        pieces = {0: t2[0:3], 1: t2[3:6], 2: t2[6:8] + t3[0:1],
                  3: t3[1:4], 4: t3[4:8]}

        for ch in range(NCHUNK):
            emit_mains(ch)
            if ch > 0:
                emit_post(ch - 1)
                for p in pieces.get(ch - 1, []):
                    p()
        emit_post(NCHUNK - 1)

        pso_ctx.__exit__(None, None, None)
        ps_ctx.__exit__(None, None, None)
        kp_ctx.__exit__(None, None, None)
        lp_ctx.__exit__(None, None, None)


_COMPILED = {}


def _get_nc(dbg: bool = False):
    key = ("nc", dbg)
    if key not in _COMPILED:
        nc = bacc.Bacc("TRN2", debug=False, num_swdge_queues=4)
        build_program(nc, dbg=dbg)
        nc.compile()
        _COMPILED[key] = nc
    return _COMPILED[key]


# slot g = 16*j + (p%16)  <->  output pixel (x = g%128, y = g//128);
# gather h covers slots [1024h, 1024(h+1)) -> partitions x, columns y.
_Jg = np.arange(GIDX)[None, :]
_Pg = np.arange(P)[:, None]
_G = 16 * _Jg + (_Pg % 16)            # [128, 1024]
_XG = (_G % 128).astype(np.int64)
_YG = (_G // 128).astype(np.int64)
_BF16 = ml_dtypes.bfloat16


def _build_tables(image_r, image_i, flow):
    """Per-timepoint: corner table (bf16, row y0*128+x0), idx16, weights."""
    ir = np.ascontiguousarray(image_r, np.float32)
    ii = np.ascontiguousarray(image_i, np.float32)
    irT, iiT = ir.T, ii.T                     # [y, x]
    y1 = np.minimum(np.arange(NX) + 1, NX - 1)
    x1 = np.minimum(np.arange(NX) + 1, NX - 1)
    tables = []
    for t in range(NT):
        f0 = np.asarray(flow[:, :, 0, t], np.float32)
        f1 = np.asarray(flow[:, :, 1, t], np.float32)
        # float32 math mirrors the jax reference exactly
        xg = np.arange(NX, dtype=np.float32)[:, None]
        yg = np.arange(NX, dtype=np.float32)[None, :]
        cx = np.clip(xg + f0, np.float32(0.0), np.float32(NX - 1))
        cy = np.clip(yg + f1, np.float32(0.0), np.float32(NX - 1))
        x0 = np.floor(cx)
        y0 = np.floor(cy)
        wx = (cx - x0).astype(np.float32)     # [x, y]
        wy = (cy - y0).astype(np.float32)
        w4 = np.stack([(1 - wx) * (1 - wy), (1 - wx) * wy,
                       wx * (1 - wy), wx * wy], axis=-1).astype(np.float32)
        x0i = x0.astype(np.int64)
        y0i = y0.astype(np.int64)
        idxv = (y0i * NX + x0i).astype(np.int16)      # [x, y]
        idx16 = idxv[_XG, _YG]                        # wrapped gather layout

        tbl = np.zeros((NX, NX, ELEM), dtype=_BF16)
        tbl[:, :, 0] = irT
        tbl[:, :, 1] = irT[y1, :]
        tbl[:, :, 2] = irT[:, x1]
        tbl[:, :, 3] = irT[y1][:, x1]
        tbl[:, :, 4] = iiT
        tbl[:, :, 5] = iiT[y1, :]
        tbl[:, :, 6] = iiT[:, x1]
        tbl[:, :, 7] = iiT[y1][:, x1]
        tables.append((tbl.reshape(NPIX, ELEM), idx16, w4))
    return tables


def make_in_maps(image_r, image_i, csm_r, csm_i, traj, dcf, flow):
    del dcf  # unused by the operator
    tables = _build_tables(image_r, image_i, flow)
    csm_r = np.ascontiguousarray(csm_r, np.float32)
    csm_i = np.ascontiguousarray(csm_i, np.float32)
    in_maps = []
    for core in range(8):
        t, h = divmod(core, 2)
        sl = slice(h * S, (h + 1) * S)
        tbl, idx16, w4 = tables[t]
        in_maps.append({
            "csm_r": csm_r,
            "csm_i": csm_i,
            "kx": np.ascontiguousarray(traj[sl, 0, t], np.float32),
            "ky": np.ascontiguousarray(traj[sl, 1, t], np.float32),
            "tbl": np.ascontiguousarray(tbl),
            "idx": np.ascontiguousarray(idx16),
            "w4": np.ascontiguousarray(w4),
        })
    return in_maps


def combine_outputs(results):
    out = np.zeros((NCOIL, NS), np.complex64)
    for core, res in enumerate(results):
        t, h = divmod(core, 2)
        sl = slice(h * S, (h + 1) * S)
        out[:, sl] += res["out_r"].astype(np.complex64) + 1j * res["out_i"].astype(
            np.complex64)
    return out


def kernel(**inputs) -> np.ndarray:
    from concourse.bass_utils import run_bass_kernel_spmd

    nc = _get_nc()
    in_maps = make_in_maps(**inputs)
    res = run_bass_kernel_spmd(nc, in_maps, core_ids=list(range(8)))
    return combine_outputs(res.results)


# revision 27
# speedup vs baseline: 1.0576x; 1.0414x over previous
"""Batchelor GPU-NUFFT forward operator on 8 Trainium2 NeuronCores.

Math (per timepoint t):
    warped  = bilinear_warp(image, flow[..., t])
    coil    = csm * warped                                  [Nc,Nx,Ny]
    out_t[c,s] = sum_{x,y} coil[c,x,y] exp(-2pi i (kx_s (x-64) + ky_s (y-64)))
    out     = sum_t out_t                                   [Nc,NS] complex64

Sharding: 8 cores = 4 timepoints x 2 sample-halves (4096 samples each).
Host unshard: sum the 4 timepoint partials per half, concat halves.

Device pipeline (per core):
  * warp: host provides the bf16 corner table (DRAM, row (y0*128+x0) holds the
    4 bilinear corners of real+imag), int16 gather indices in the SWDGE
    wrapped layout, and the 4 bilinear weight planes. 16 dma_gather ops land
    the corners directly in [x, y] layout (slot i = y*128 + x); the combine
    and the coil pack run per 4-gather quarter in the gather shadow.
  * NUFFT: Khatri-Rao split y = yo*8 + yi. Per 512-sample chunk, 32
    accumulating bf16 matmuls build PSUM partials Pr = Re(sum coil e^{-iA}),
    Pi = Im(...) directly (stationary blocks Cr | Ci | -Ci make the +- signs
    accumulate in PSUM). The outer phase e^{-iB} is 4 elementwise products,
    folded to 8 coils by +-selector matmuls.
  * trig: phases are range-reduced with custom DVE ops (PHASE_WRAP fuses the
    a0 = kx*(x-64) - 64*ky wrap to one op; ADD_TT_WRAP fuses each chain step
    m2_yi = wrap(m2_parent + ky2^j) with a log-depth parent tree; ABS_SUB
    preps cos args as |m|-1/4 since the ACT Sin spline is only valid on
    [-pi, pi]). ky2/ky4 = wrap(2ky), wrap(4ky) come from the host. All four
    1024-wide trig batches are emitted ahead of the MM loop so the Scalar
    engine streams Sin evaluations while the gather runs.
"""

import sys

if "/opt/trn_rl_repo" not in sys.path:
    sys.path.insert(0, "/opt/trn_rl_repo")

import math

import numpy as np
import ml_dtypes

import concourse.bass as bass
import concourse.tile as tile
from concourse import bacc
from concourse import mybir
from concourse import dve_ops
from concourse.dve_spec import Spec, Src0, Src1, C0, C1, C2, Zero, maxx

P = 128
NX = 128
NCOIL = 8
NS = 8192
NT = 4
S = 4096           # samples per core (half of NS)
CH = 512           # samples per MM chunk (PSUM bank width)
NCHUNK = S // CH   # 8
BW = 1024          # trig batch width (2 chunks)
NBATCH = S // BW   # 4
YI = 8
YO = 16
NPIX = NX * NX
NGATH = 16
GIDX = NPIX // NGATH   # 1024 indices per gather
ELEM = 128             # bf16 elements per table row = 256 bytes
NABS_SC = 4            # yi < NABS_SC: cos-prep via scalar Abs; else DVE ABS_SUB
GP_OUTER = 0           # gpsimd cannot read PSUM: outer products stay on DVE

F32 = mybir.dt.float32
BF16 = mybir.dt.bfloat16
I16 = mybir.dt.int16
TWO_PI = float(2.0 * math.pi)
MAGIC = 12582912.0  # 1.5*2^23: (x + M) - M == round-to-nearest(x) for f32
ALU = mybir.AluOpType
ACTF = mybir.ActivationFunctionType


# ---------------- custom DVE ops ----------------
def _register_dve_op(name, spec):
    if name in dve_ops._SUB_OPCODE_FOR_NAME:
        for op in dve_ops.OPS:
            if op.name == name:
                return op
        raise RuntimeError(name)
    shas = {}
    for ver in ("v3", "v4"):
        uops = dve_ops.lower(spec, ver=ver)
        shas[ver] = dve_ops.DveOpSpec(
            name=name, opcode=0, uops=uops, rd1_en=dve_ops.has_src1(spec)
        ).sha(ver)
    op = dve_ops.DveOp(name, spec, subdim=False, uops_sha=shas)
    dve_ops.OPS.append(op)
    dve_ops._SUB_OPCODE_FOR_NAME[name] = (
        dve_ops._CUSTOM_DVE_ROW_BASE + len(dve_ops.OPS) - 1
    )
    dve_ops.CUSTOM_DVE_SPECS[name] = spec
    return op


def _wrap_np(v):
    return (v - np.round(v)).astype(np.float32)


# out = m - round(m), m = in0*s0 + in1*s1  (s0 may be a [P,1] AP)
_pw_m = Src0 * C0 + Src1 * C1
_pw_r = (_pw_m + C2) - C2
PHASE_WRAP = _register_dve_op(
    "PHASE_WRAP_ANT",
    Spec(
        body=_pw_m - _pw_r,
        reference=lambda in0, in1, s0, s1, imm2: (
            (in0 * s0 + in1 * s1)
            - (((in0 * s0 + in1 * s1) + imm2) - imm2)
        ).astype(np.float32),
    ),
)

# out = y - ((y > .5) - (y < -.5)), y = in0 + in1 : one-period wrap of a sum
_aw_y = Src0 + Src1
ADD_TT_WRAP = _register_dve_op(
    "ADD_TT_WRAP_ANT",
    Spec(
        body=_aw_y + C2 * ((_aw_y < (Zero - C1)) - (C1 < _aw_y)),
        reference=lambda in0, in1, s0, s1, imm2: (
            (in0 + in1)
            + imm2
            * (
                ((in0 + in1) < -s1).astype(np.float32)
                - ((in0 + in1) > s1).astype(np.float32)
            )
        ).astype(np.float32),
    ),
)

# out = |in0| + s0
ABS_SUB = _register_dve_op(
    "ABS_ADD_ANT",
    Spec(
        body=maxx(Src0, Zero - Src0) + C0,
        reference=lambda in0, in1, s0, s1, imm2: (np.abs(in0) + s0).astype(
            np.float32
        ),
    ),
)


def build_program(nc: bass.Bass, dbg: bool = False):
    def dbg_out(name, src_ap, shape, dtype=F32):
        if not dbg:
            return
        d = nc.dram_tensor("dbg_" + name, shape, dtype, kind="ExternalOutput").ap()
        nc.sync.dma_start(d[:], src_ap)

    csm_r = nc.dram_tensor("csm_r", [NCOIL, NX, NX], F32, kind="ExternalInput").ap()
    csm_i = nc.dram_tensor("csm_i", [NCOIL, NX, NX], F32, kind="ExternalInput").ap()
    kx_d = nc.dram_tensor("kx", [S], F32, kind="ExternalInput").ap()
    ky_d = nc.dram_tensor("ky", [S], F32, kind="ExternalInput").ap()
    tbl_d = nc.dram_tensor("tbl", [NPIX, ELEM], BF16, kind="ExternalInput").ap()
    idx_d = nc.dram_tensor("idx", [P, GIDX], I16, kind="ExternalInput").ap()
    w4_d = nc.dram_tensor("w4", [P, NX, 4], F32, kind="ExternalInput").ap()
    out_r = nc.dram_tensor("out_r", [NCOIL, S], F32, kind="ExternalOutput").ap()
    out_i = nc.dram_tensor("out_i", [NCOIL, S], F32, kind="ExternalOutput").ap()

    # ---------------- inline constants ----------------
    pvals = np.arange(P, dtype=np.float32)
    xc_d = nc.inline_tensor((pvals - 64.0).reshape(P, 1), name="c_xc").ap()
    yo8_d = nc.inline_tensor((8.0 * (np.arange(P) % 16)).astype(np.float32)
                             .reshape(P, 1), name="c_yo8").ap()
    half_pi_d = nc.inline_tensor(np.full((P, 1), math.pi / 2, np.float32),
                                 name="c_half_pi").ap()
    sel_np = (np.arange(P)[:, None] // 16 == np.arange(NCOIL)[None, :]).astype(
        np.float32)
    selpm_np = np.concatenate([sel_np, -sel_np], axis=1)  # [128, 16]: +sel | -sel
    selpm_d = nc.inline_tensor(selpm_np, name="c_selpm").ap()

    with tile.TileContext(nc) as tc, \
         tc.tile_pool(name="pp", bufs=1) as pp:

        # --- persistent constants / inputs ---
        idx16 = pp.tile([P, GIDX], I16)
        H = S // 2
        kxb = pp.tile([P, S], F32)
        nc.scalar.dma_start(
            kxb[:, 0:H],
            kx_d[0:H].rearrange("(p s) -> p s", p=1).to_broadcast([P, H]))
        kyb = pp.tile([P, S], F32)
        nc.sync.dma_start(
            kyb[:, 0:H],
            ky_d[0:H].rearrange("(p s) -> p s", p=1).to_broadcast([P, H]))
        xc_col = pp.tile([P, 1], F32)
        nc.sync.dma_start(xc_col[:], xc_d[:])
        yo8 = pp.tile([P, 1], F32)
        nc.sync.dma_start(yo8[:], yo8_d[:])
        half_pi = pp.tile([P, 1], F32)
        nc.sync.dma_start(half_pi[:], half_pi_d[:])
        selpm32 = pp.tile([P, 2 * NCOIL], F32)
        nc.sync.dma_start(selpm32[:], selpm_d[:])
        selpm = pp.tile([P, 2 * NCOIL], BF16)
        nc.vector.tensor_copy(selpm[:], selpm32[:])

        # packed coil stationary: blocks [Cr | Ci | -Ci], col = c*16 + yo,
        # innermost yi so the pack writes contiguous 16B runs
        RA = pp.tile([P, 3, P, YI], BF16)

        # --- pools (gp innermost so it can close after the warp) ---
        lp_ctx = tc.tile_pool(name="loop", bufs=1)
        lp = lp_ctx.__enter__()
        kp_ctx = tc.tile_pool(name="kr", bufs=1)
        kp = kp_ctx.__enter__()
        gp_pool_ctx = tc.tile_pool(name="gp", bufs=1)
        gp = gp_pool_ctx.__enter__()
        g8p = gp.tile([P, NX, ELEM], BF16)
        w4sb = gp.tile([P, NX, 4], F32)
        nc.scalar.dma_start(w4sb[:], w4_d[:])
        csm_r_sb = gp.tile([P, NCOIL, NX], F32)
        nc.sync.dma_start(csm_r_sb[:], csm_r.rearrange("c x y -> x c y"))
        csm_i_sb = gp.tile([P, NCOIL, NX], F32)
        nc.sync.dma_start(csm_i_sb[:], csm_i.rearrange("c x y -> x c y"))

        # idx16 is loaded LAST on the sync queue: HWDGE executes FIFO, so the
        # gathers (which depend on idx16) cannot start stealing SDMA slots
        # until every other input DMA has landed.
        nc.sync.dma_start(idx16[:], idx_d[:])
        nc.sync.dma_start(
            kxb[:, H:S],
            kx_d[H:S].rearrange("(p s) -> p s", p=1).to_broadcast([P, S - H]))
        nc.sync.dma_start(
            kyb[:, H:S],
            ky_d[H:S].rearrange("(p s) -> p s", p=1).to_broadcast([P, S - H]))
        gsems = [nc.alloc_semaphore(f"gath_sem{q}") for q in range(4)]
        for h in range(NGATH):
            nc.gpsimd.dma_gather(
                out_ap=g8p[:, h * 8:(h + 1) * 8, :],
                in_ap=tbl_d[:],
                idxs_ap=idx16[:, h * 64:(h + 1) * 64],
                num_idxs=GIDX,
                num_idxs_reg=GIDX,
                elem_size=ELEM,
                queue_num=h % 4,
            ).then_inc(gsems[h % 4], 16)

        # ---------------- trig batches (emitted in pieces) ----------------
        trig = {}

        def make_trig(b):
            cs = slice(b * BW, (b + 1) * BW)
            nabs = 2 if b == 3 else NABS_SC
            st = {"m2": {}, "kits": [], "krts": []}

            def emit_yi(yi):
                m2 = st["m2"]
                kyc = st["kyc"]
                if yi > 0:
                    t = lp.tile([P, BW], F32, tag="m2c", bufs=2)
                    nc.vector._custom_dve(ADD_TT_WRAP, out=t[:],
                                          in0=m2[yi - 1][:],
                                          in1=kyc, s1=0.5, imm2=1.0)
                    m2[yi] = t
                kit = kp.tile([P, BW], BF16, tag=f"kit{yi}", bufs=2)
                nc.scalar.activation(kit[:], m2[yi][:], ACTF.Sin, scale=-TWO_PI)
                krt = kp.tile([P, BW], BF16, tag=f"krt{yi}", bufs=2)
                if yi < nabs:
                    mabs = lp.tile([P, BW], F32, tag="mabs", bufs=1)
                    nc.scalar.activation(mabs[:], m2[yi][:], ACTF.Abs)
                    nc.scalar.activation(krt[:], mabs[:], ACTF.Sin,
                                         scale=-TWO_PI, bias=half_pi[:, 0:1])
                else:
                    mk = lp.tile([P, BW], F32, tag="mk", bufs=2)
                    nc.vector._custom_dve(ABS_SUB, out=mk[:], in0=m2[yi][:],
                                          s0=-0.25)
                    nc.scalar.activation(krt[:], mk[:], ACTF.Sin, scale=-TWO_PI)
                st["kits"].append(kit)
                st["krts"].append(krt)

            def piece0():
                kxc = kxb[:, cs]
                kyc = st["kyc"] = kyb[:, cs]
                m2o = lp.tile([P, BW], F32, tag="m2o", bufs=1)
                nc.vector._custom_dve(PHASE_WRAP, out=m2o[:], in0=kyc,
                                      in1=kyc, s0=yo8[:, 0:1], s1=0.0,
                                      imm2=MAGIC)
                mok = lp.tile([P, BW], F32, tag="mok", bufs=1)
                nc.vector._custom_dve(ABS_SUB, out=mok[:], in0=m2o[:], s0=-0.25)
                aic = kp.tile([P, BW], BF16, tag="aic", bufs=2)
                nc.scalar.activation(aic[:], m2o[:], ACTF.Sin, scale=-TWO_PI)
                arc = kp.tile([P, BW], BF16, tag="arc", bufs=2)
                nc.scalar.activation(arc[:], mok[:], ACTF.Sin, scale=-TWO_PI)
                m2a = lp.tile([P, BW], F32, tag="m2a", bufs=1)
                nc.vector._custom_dve(PHASE_WRAP, out=m2a[:], in0=kxc,
                                      in1=kyc, s0=xc_col[:, 0:1], s1=-64.0,
                                      imm2=MAGIC)
                st["m2"][0] = m2a
                emit_yi(0)
                trig[b] = (st["kits"], st["krts"], arc, aic)

            return [piece0] + [lambda yi=yi: emit_yi(yi) for yi in range(1, YI)]

        # ---------------- warp eighth: combine + pack ----------------
        def emit_quarter(q):
            W = 16
            ys = slice(W * q, W * q + W)
            for h in (2 * q, 2 * q + 1):
                nc.vector.wait_ge(gsems[h % 4], 16 * (h // 4 + 1))
            t8r = gp.tile([P, W, 4], F32, tag="t8r", bufs=2)
            nc.vector.tensor_tensor(t8r[:], g8p[:, ys, 0:4], w4sb[:, ys, :],
                                    op=ALU.mult)
            warped_r = gp.tile([P, W], F32, tag="wr", bufs=2)
            nc.vector.reduce_sum(warped_r[:], t8r[:], axis=mybir.AxisListType.X)
            t8i = gp.tile([P, W, 4], F32, tag="t8i", bufs=2)
            nc.vector.tensor_tensor(t8i[:], g8p[:, ys, 4:8], w4sb[:, ys, :],
                                    op=ALU.mult)
            warped_i = gp.tile([P, W], F32, tag="wi", bufs=2)
            nc.vector.reduce_sum(warped_i[:], t8i[:], axis=mybir.AxisListType.X)

            wr_b = warped_r[:].rearrange("p (c y) -> p c y", c=1).to_broadcast(
                [P, NCOIL, W])
            wi_b = warped_i[:].rearrange("p (c y) -> p c y", c=1).to_broadcast(
                [P, NCOIL, W])
            csr = csm_r_sb[:, :, ys]
            csi = csm_i_sb[:, :, ys]

            # RA views for this eighth: [p, c, yo(2), yi(8)], contiguous yi
            ra5 = RA[:].rearrange("p b (c yo) yi -> p b c yo yi", c=NCOIL)
            NYO = W // YI
            cr_v = ra5[:, 0, :, NYO * q:NYO * q + NYO, :]
            ci_v = ra5[:, 1, :, NYO * q:NYO * q + NYO, :]
            cin_v = ra5[:, 2, :, NYO * q:NYO * q + NYO, :]

            def as4(t):
                return t.rearrange("p c (yo yi) -> p c yo yi", yi=YI)

            tt1 = gp.tile([P, NCOIL, W], F32, tag="tt1", bufs=2)
            nc.vector.tensor_tensor(tt1[:], csr, wr_b, op=ALU.mult)
            tt2 = gp.tile([P, NCOIL, W], F32, tag="tt2", bufs=2)
            nc.vector.tensor_tensor(tt2[:], csi, wi_b, op=ALU.mult)
            nc.vector.tensor_tensor(cr_v, as4(tt1[:]), as4(tt2[:]),
                                    op=ALU.subtract)
            tt3 = gp.tile([P, NCOIL, W], F32, tag="tt1", bufs=2)
            nc.vector.tensor_tensor(tt3[:], csr, wi_b, op=ALU.mult)
            tt4 = gp.tile([P, NCOIL, W], F32, tag="tt2", bufs=2)
            nc.vector.tensor_tensor(tt4[:], csi, wr_b, op=ALU.mult)
            cit = gp.tile([P, NCOIL, W], F32, tag="cit", bufs=2)
            nc.vector.tensor_tensor(cit[:], tt3[:], tt4[:], op=ALU.add)
            nc.vector.tensor_copy(ci_v, as4(cit[:]))
            nc.vector.tensor_scalar(cin_v, as4(cit[:]), -1.0, None, op0=ALU.mult)

        # ---------------- MM chunk ----------------
        ps_ctx = tc.tile_pool(name="ps", bufs=1, space="PSUM")
        ps = ps_ctx.__enter__()
        pso_ctx = tc.tile_pool(name="pso", bufs=1, space="PSUM")
        pso = pso_ctx.__enter__()

        live = {}

        def emit_mains(ch):
            b, half = divmod(ch, 2)
            sl = slice(half * CH, (half + 1) * CH)
            kits, krts, arc, aic = trig[b]
            Pr = ps.tile([P, CH], F32, tag="Pr", bufs=3)
            Pi = ps.tile([P, CH], F32, tag="Pi", bufs=3)
            for yi in range(YI):
                st, sp = (yi == 0), (yi == YI - 1)
                krt_s = krts[yi][:, sl]
                kit_s = kits[yi][:, sl]
                nc.tensor.matmul(Pr[:], RA[:, 0, :, yi], krt_s,
                                 start=st, stop=False)
                nc.tensor.matmul(Pi[:], RA[:, 0, :, yi], kit_s,
                                 start=st, stop=False)
                nc.tensor.matmul(Pr[:], RA[:, 2, :, yi], kit_s,
                                 start=False, stop=sp)
                nc.tensor.matmul(Pi[:], RA[:, 1, :, yi], krt_s,
                                 start=False, stop=sp)
            live[ch] = (Pr, Pi, arc, aic, sl)

        def emit_post(ch):
            c0 = ch * CH
            Pr, Pi, arc, aic, sl = live.pop(ch)
            q1 = lp.tile([P, CH], BF16, tag="q1", bufs=2)
            nc.vector.tensor_tensor(q1[:], Pr[:], arc[:, sl], op=ALU.mult)
            q2 = lp.tile([P, CH], BF16, tag="q2", bufs=2)
            nc.vector.tensor_tensor(q2[:], Pi[:], aic[:, sl], op=ALU.mult)
            eng3 = nc.gpsimd if GP_OUTER >= 1 else nc.vector
            eng4 = nc.gpsimd if GP_OUTER >= 2 else nc.vector
            q3 = lp.tile([P, CH], BF16, tag="q3", bufs=2)
            eng3.tensor_tensor(q3[:], Pi[:], arc[:, sl], op=ALU.mult)
            q4 = lp.tile([P, CH], BF16, tag="q4", bufs=2)
            eng4.tensor_tensor(q4[:], Pr[:], aic[:, sl], op=ALU.mult)

            SP, SM = selpm[:, 0:NCOIL], selpm[:, NCOIL:2 * NCOIL]
            po = pso.tile([32 + NCOIL, CH], F32, tag="po", bufs=2)
            nc.tensor.matmul(po[0:NCOIL], SP, q1[:], start=True, stop=False)
            nc.tensor.matmul(po[0:NCOIL], SM, q2[:], start=False, stop=True)
            nc.tensor.matmul(po[32:32 + NCOIL], SP, q3[:], start=True,
                             stop=False)
            nc.tensor.matmul(po[32:32 + NCOIL], SP, q4[:], start=False,
                             stop=True)
            ost = lp.tile([32 + NCOIL, CH], F32, tag="ost", bufs=2)
            if ch < 4:
                nc.scalar.copy(ost[:], po[:])
            else:
                nc.vector.tensor_copy(ost[:], po[:])
            nc.sync.dma_start(out_r[:, c0:c0 + CH], ost[0:NCOIL])
            nc.sync.dma_start(out_i[:, c0:c0 + CH], ost[32:32 + NCOIL])

        # ---------------- emission schedule ----------------
        for p in make_trig(0):
            p()
        for q in range(8):
            emit_quarter(q)
        dbg_out("RA", RA[:].rearrange("p b c yi -> p (b c yi)"), [P, YI * 3 * P],
                BF16)
        gp_pool_ctx.__exit__(None, None, None)
        for p in make_trig(1):
            p()

        # interleave trig batches 2/3 into the MM loop so the DVE queue never
        # blocks the selector matmuls: b2 pieces land after posts 1-3, b3
        # after posts 3-5.
        t2 = make_trig(2)
        t3 = make_trig(3)
        pieces = {0: t2[0:4], 1: t2[4:8], 2: t3[0:3],
                  3: t3[3:6], 4: t3[6:8]}

        for ch in range(NCHUNK):
            emit_mains(ch)
            if ch > 0:
                emit_post(ch - 1)
                for p in pieces.get(ch - 1, []):
                    p()
        emit_post(NCHUNK - 1)

        pso_ctx.__exit__(None, None, None)
        ps_ctx.__exit__(None, None, None)
        kp_ctx.__exit__(None, None, None)
        lp_ctx.__exit__(None, None, None)


_COMPILED = {}


def _get_nc(dbg: bool = False):
    key = ("nc", dbg)
    if key not in _COMPILED:
        nc = bacc.Bacc("TRN2", debug=False, num_swdge_queues=4)
        build_program(nc, dbg=dbg)
        nc.compile()
        _COMPILED[key] = nc
    return _COMPILED[key]


# slot g = 16*j + (p%16)  <->  output pixel (x = g%128, y = g//128);
# gather h covers slots [1024h, 1024(h+1)) -> partitions x, columns y.
_Jg = np.arange(GIDX)[None, :]
_Pg = np.arange(P)[:, None]
_G = 16 * _Jg + (_Pg % 16)            # [128, 1024]
_XG = (_G % 128).astype(np.int64)
_YG = (_G // 128).astype(np.int64)
_BF16 = ml_dtypes.bfloat16


def _build_tables(image_r, image_i, flow):
    """Per-timepoint: corner table (bf16, row y0*128+x0), idx16, weights."""
    ir = np.ascontiguousarray(image_r, np.float32)
    ii = np.ascontiguousarray(image_i, np.float32)
    irT, iiT = ir.T, ii.T                     # [y, x]
    y1 = np.minimum(np.arange(NX) + 1, NX - 1)
    x1 = np.minimum(np.arange(NX) + 1, NX - 1)
    tables = []
    for t in range(NT):
        f0 = np.asarray(flow[:, :, 0, t], np.float32)
        f1 = np.asarray(flow[:, :, 1, t], np.float32)
        # float32 math mirrors the jax reference exactly
        xg = np.arange(NX, dtype=np.float32)[:, None]
        yg = np.arange(NX, dtype=np.float32)[None, :]
        cx = np.clip(xg + f0, np.float32(0.0), np.float32(NX - 1))
        cy = np.clip(yg + f1, np.float32(0.0), np.float32(NX - 1))
        x0 = np.floor(cx)
        y0 = np.floor(cy)
        wx = (cx - x0).astype(np.float32)     # [x, y]
        wy = (cy - y0).astype(np.float32)
        w4 = np.stack([(1 - wx) * (1 - wy), (1 - wx) * wy,
                       wx * (1 - wy), wx * wy], axis=-1).astype(np.float32)
        x0i = x0.astype(np.int64)
        y0i = y0.astype(np.int64)
        idxv = (y0i * NX + x0i).astype(np.int16)      # [x, y]
        idx16 = idxv[_XG, _YG]                        # wrapped gather layout

        tbl = np.zeros((NX, NX, ELEM), dtype=_BF16)
        tbl[:, :, 0] = irT
        tbl[:, :, 1] = irT[y1, :]
        tbl[:, :, 2] = irT[:, x1]
        tbl[:, :, 3] = irT[y1][:, x1]
        tbl[:, :, 4] = iiT
        tbl[:, :, 5] = iiT[y1, :]
        tbl[:, :, 6] = iiT[:, x1]
        tbl[:, :, 7] = iiT[y1][:, x1]
        tables.append((tbl.reshape(NPIX, ELEM), idx16, w4))
    return tables


def make_in_maps(image_r, image_i, csm_r, csm_i, traj, dcf, flow):
    del dcf  # unused by the operator
    tables = _build_tables(image_r, image_i, flow)
    csm_r = np.ascontiguousarray(csm_r, np.float32)
    csm_i = np.ascontiguousarray(csm_i, np.float32)
    in_maps = []
    for core in range(8):
        t, h = divmod(core, 2)
        sl = slice(h * S, (h + 1) * S)
        tbl, idx16, w4 = tables[t]
        in_maps.append({
            "csm_r": csm_r,
            "csm_i": csm_i,
            "kx": np.ascontiguousarray(traj[sl, 0, t], np.float32),
            "ky": np.ascontiguousarray(traj[sl, 1, t], np.float32),
            "tbl": np.ascontiguousarray(tbl),
            "idx": np.ascontiguousarray(idx16),
            "w4": np.ascontiguousarray(w4),
        })
    return in_maps


def combine_outputs(results):
    out = np.zeros((NCOIL, NS), np.complex64)
    for core, res in enumerate(results):
        t, h = divmod(core, 2)
        sl = slice(h * S, (h + 1) * S)
        out[:, sl] += res["out_r"].astype(np.complex64) + 1j * res["out_i"].astype(
            np.complex64)
    return out


def kernel(**inputs) -> np.ndarray:
    from concourse.bass_utils import run_bass_kernel_spmd

    nc = _get_nc()
    in_maps = make_in_maps(**inputs)
    res = run_bass_kernel_spmd(nc, in_maps, core_ids=list(range(8)))
    return combine_outputs(res.results)
